# revision 19
# baseline (speedup 1.0000x reference)
"""Trainium2 Bass kernel for nn_MultiHeadAttention_46471546143554.

Head-parallel sharding: 16 heads / 8 cores = 2 heads per core. Each core
computes QKV projection (its head slice), RoPE, causal attention, and a
per-head output projection producing a partial [B*T, C] sum; the host adds
the 8 partials.

Layout trick: everything runs "transposed" ([feature, token]) so the PE
contracts over partitions with zero on-device transposes of activations:
  qkvT = W.T @ xT          (xT passed pre-transposed from host)
  S^T  = kT.T @ qT         (per 128-key block)
  P^T  = exp(S^T * scale)  (no max subtraction; scores are O(+-8))
  A^T  = v_aug.T @ P^T     (v_aug = [v | ones] -> row 64 = softmax denom)
  out  = A^T.T @ Wp_head   (per head; divide by denom at PSUM eviction,
                            where the denom is a per-partition scalar)
"""
import numpy as np

import concourse.bass as bass
import concourse.mybir as mybir
import concourse.tile as tile
from concourse import bacc
from concourse import bass_utils

B, T, C = 2, 2048, 1024
H, HD, HALF = 16, 64, 32
BT = B * T
N_CORES = 8
HPC = 2              # heads per core
NKC = C // 128       # contraction chunks for projection
NJ = BT // 512       # 512-token blocks overall
NQ = T // 512        # tq blocks per batch
NKB = T // 128       # tk blocks per batch
ROPE_BASE = 10000.0

F32 = mybir.dt.float32
F32R = mybir.dt.float32r
MM_DT = F32R         # matmul streaming dtype (1 cycle/row when N>=256)
SDT = MM_DT          # storage dtype for tiles feeding matmuls
SCALE = float(HD) ** -0.5


def _mm(ap):
    return ap


def build_program(nc):
    xT = nc.dram_tensor("xT", [C, BT], SDT, kind="ExternalInput").ap()
    wq = nc.dram_tensor("wq", [C, 128], SDT, kind="ExternalInput").ap()
    wk = nc.dram_tensor("wk", [C, 128], SDT, kind="ExternalInput").ap()
    wv = nc.dram_tensor("wv", [C, 128], SDT, kind="ExternalInput").ap()
    wp = nc.dram_tensor("wp", [128, C], SDT, kind="ExternalInput").ap()
    cb = nc.dram_tensor("cb", [128, T], SDT, kind="ExternalInput").ap()
    sb = nc.dram_tensor("sb", [128, T], SDT, kind="ExternalInput").ap()
    perm = nc.dram_tensor("perm", [128, 128], SDT, kind="ExternalInput").ap()
    tri = nc.dram_tensor("tri", [128, 128], SDT, kind="ExternalInput").ap()
    idt = nc.dram_tensor("idt", [128, 128], SDT, kind="ExternalInput").ap()
    ones = nc.dram_tensor("ones", [128, HPC, NJ * 4, 64], SDT,
                          kind="ExternalInput").ap()
    out = nc.dram_tensor("out", [BT, C], F32, kind="ExternalOutput").ap()

    with tile.TileContext(nc) as tc:
        from contextlib import ExitStack
        with ExitStack() as ctx:
            const = ctx.enter_context(tc.tile_pool(name="const", bufs=1))
            persist = ctx.enter_context(tc.tile_pool(name="persist", bufs=1))

            wq_s = const.tile([128, NKC, 128], SDT, tag="wq")
            wk_s = const.tile([128, NKC, 128], SDT, tag="wk")
            wv_s = const.tile([128, NKC, 128], SDT, tag="wv")
            wp_s = const.tile([64, HPC, C], SDT, tag="wp")
            cb_s = const.tile([128, T], SDT, tag="cb")
            sb_s = const.tile([128, T], SDT, tag="sb")
            perm_s = const.tile([128, 128], SDT, tag="perm")
            tri_s = const.tile([128, 128], SDT, tag="tri")
            idt_s = const.tile([128, 128], SDT, tag="idt")
            for dst, src in ((wq_s, wq), (wk_s, wk), (wv_s, wv)):
                nc.sync.dma_start(dst[:], src.rearrange("(kc p) m -> p kc m", p=128))
            nc.sync.dma_start(wp_s[:], wp.rearrange("(h p) c -> p h c", h=HPC))
            nc.sync.dma_start(cb_s[:], cb[:])
            nc.sync.dma_start(sb_s[:], sb[:])
            nc.sync.dma_start(perm_s[:], perm[:])
            nc.sync.dma_start(tri_s[:], tri[:])
            nc.sync.dma_start(idt_s[:], idt[:])

            qT_s = persist.tile([128, BT], SDT, tag="qT")
            kT_s = persist.tile([128, BT], SDT, tag="kT")
            # v_aug[:, h, kbg, :]: [v_nat | ones] per head, per global 128-key blk
            vag_s = persist.tile([128, HPC, NJ * 4, 128], SDT, tag="vag")
            nc.sync.dma_start(vag_s[:, :, :, 64:128], ones[:])

            # ---------------- Phase 1: QKV projection + RoPE -------------
            with (
                tc.tile_pool(name="xp", bufs=3) as xp,
                tc.tile_pool(name="evp", bufs=3) as evp,
                tc.tile_pool(name="rtmp", bufs=4) as rtmp,
                tc.tile_pool(name="psA", bufs=2, space="PSUM") as psA,
                tc.tile_pool(name="psB", bufs=2, space="PSUM") as psB,
                tc.tile_pool(name="psC", bufs=2, space="PSUM") as psC,
                tc.tile_pool(name="psVT", bufs=1, space="PSUM") as psVT,
            ):
                for j in range(NJ):
                    js = slice(j * 512, (j + 1) * 512)
                    rs_ = slice((j % NQ) * 512, (j % NQ + 1) * 512)  # rope cols
                    ps_q = psA.tile([128, 512], F32, tag="ps_q")
                    ps_k = psB.tile([128, 512], F32, tag="ps_k")
                    ps_v = psC.tile([128, 512], F32, tag="ps_v")
                    for kc in range(NKC):
                        xc = xp.tile([128, 512], SDT, tag="xc")
                        nc.sync.dma_start(xc[:], xT[kc * 128:(kc + 1) * 128, js])
                        st, sp = kc == 0, kc == NKC - 1
                        nc.tensor.matmul(ps_q[:], _mm(wq_s[:, kc, :]), _mm(xc[:]),
                                         start=st, stop=sp)
                        nc.tensor.matmul(ps_k[:], _mm(wk_s[:, kc, :]), _mm(xc[:]),
                                         start=st, stop=sp)
                        nc.tensor.matmul(ps_v[:], _mm(wv_s[:, kc, :]), _mm(xc[:]),
                                         start=st, stop=sp)
                    # ---- v: transpose to natural layout, augment with ones
                    vtmp = evp.tile([128, 512], SDT, tag="vtmp")
                    nc.scalar.copy(vtmp[:], ps_v[:])
                    for h in range(HPC):
                        for t4 in range(4):
                            ps_vt = psVT.tile([128, 64], SDT, tag="ps_vt")
                            nc.tensor.transpose(
                                ps_vt[:],
                                vtmp[h * 64:(h + 1) * 64, t4 * 128:(t4 + 1) * 128],
                                idt_s[h * 64:(h + 1) * 64, h * 64:(h + 1) * 64])
                            nc.scalar.copy(vag_s[:, h, j * 4 + t4, 0:64], ps_vt[:])
                    # ---- q, k: RoPE (qr = q*CB + (PERM.T @ q)*SB)
                    for ps_x, dstT in ((ps_q, qT_s), (ps_k, kT_s)):
                        raw = evp.tile([128, 512], SDT, tag="raw")
                        nc.scalar.copy(raw[:], ps_x[:])
                        ps_sw = psVT.tile([128, 512], F32, tag="ps_sw")
                        nc.tensor.matmul(ps_sw[:], _mm(perm_s[:]), _mm(raw[:]),
                                         start=True, stop=True)
                        t1 = rtmp.tile([128, 512], SDT, tag="t1")
                        t2 = rtmp.tile([128, 512], SDT, tag="t2")
                        nc.vector.tensor_mul(t1[:], ps_sw[:], sb_s[:, rs_])
                        nc.vector.tensor_mul(t2[:], raw[:], cb_s[:, rs_])
                        nc.vector.tensor_add(dstT[:, js], t1[:], t2[:])

            # ---------------- Phase 2: attention + output projection -----
            with (
                tc.tile_pool(name="pp", bufs=4) as pp,
                tc.tile_pool(name="atp", bufs=4) as atp,
                tc.tile_pool(name="rcp", bufs=4) as rcp,
                tc.tile_pool(name="otp", bufs=3) as otp,
                tc.tile_pool(name="psS", bufs=2, space="PSUM") as psS,
                tc.tile_pool(name="psAT", bufs=2, space="PSUM") as psAT,
                tc.tile_pool(name="psRS", bufs=1, space="PSUM") as psRS,
                tc.tile_pool(name="psO", bufs=3, space="PSUM") as psO,
            ):
                for b in range(B):
                    for jq in range(NQ):
                        att65 = []
                        rc = []
                        for h in range(HPC):
                            hs = slice(h * 64, (h + 1) * 64)
                            ps_at = psAT.tile([128, 512], F32, tag="ps_at")
                            nkb = 4 * jq + 4
                            for kb in range(nkb):
                                kcols = slice(b * T + kb * 128,
                                              b * T + (kb + 1) * 128)
                                c0 = max((kb - 4 * jq) * 128, 0)
                                qcols_t = slice(b * T + jq * 512 + c0,
                                                b * T + (jq + 1) * 512)
                                ps_s = psS.tile([128, 512], F32, tag="ps_s")
                                nc.tensor.matmul(ps_s[:, c0:512],
                                                 _mm(kT_s[hs, kcols]),
                                                 _mm(qT_s[hs, qcols_t]),
                                                 start=True, stop=True)
                                pt = pp.tile([128, 512], SDT, tag="pt")
                                nc.scalar.activation(
                                    pt[:, c0:512], ps_s[:, c0:512],
                                    mybir.ActivationFunctionType.Exp,
                                    scale=SCALE)
                                if kb >= 4 * jq:
                                    nc.vector.tensor_mul(
                                        pt[:, c0:c0 + 128],
                                        pt[:, c0:c0 + 128], tri_s[:])
                                nc.tensor.matmul(
                                    ps_at[:, c0:512],
                                    _mm(vag_s[:, h, b * 16 + kb, :]),
                                    _mm(pt[:, c0:512]),
                                    start=(kb == 0), stop=(kb == nkb - 1))
                            a65 = atp.tile([66, 512], SDT, tag="a65")
                            nc.scalar.copy(a65[:], ps_at[0:66, :])
                            rch = rcp.tile([128, 4], F32, tag="rch")
                            for t4 in range(4):
                                ps_rs = psRS.tile([128, 2], SDT, tag="ps_rs")
                                nc.tensor.transpose(
                                    ps_rs[:],
                                    a65[64:66, t4 * 128:(t4 + 1) * 128],
                                    idt_s[64:66, 64:66])
                                nc.vector.reciprocal(rch[:, t4:t4 + 1],
                                                     ps_rs[:, 0:1])
                            att65.append(a65)
                            rc.append(rch)
                        # ---- output projection, fused per-head denominators
                        for t4 in range(4):
                            rows = slice(b * T + jq * 512 + t4 * 128,
                                         b * T + jq * 512 + (t4 + 1) * 128)
                            for n2 in range(2):
                                ns = slice(n2 * 512, (n2 + 1) * 512)
                                ps_o0 = psO.tile([128, 512], F32, tag="ps_o")
                                ps_o1 = psO.tile([128, 512], F32, tag="ps_o")
                                nc.tensor.matmul(
                                    ps_o0[:],
                                    _mm(att65[0][0:64, t4 * 128:(t4 + 1) * 128]),
                                    _mm(wp_s[:, 0, ns]), start=True, stop=True)
                                nc.tensor.matmul(
                                    ps_o1[:],
                                    _mm(att65[1][0:64, t4 * 128:(t4 + 1) * 128]),
                                    _mm(wp_s[:, 1, ns]), start=True, stop=True)
                                ot = otp.tile([128, 512], F32, tag="ot")
                                nc.scalar.activation(
                                    ot[:], ps_o0[:],
                                    mybir.ActivationFunctionType.Copy,
                                    scale=rc[0][:, t4:t4 + 1])
                                nc.vector.scalar_tensor_tensor(
                                    ot[:], ps_o1[:], rc[1][:, t4:t4 + 1], ot[:],
                                    op0=mybir.AluOpType.mult,
                                    op1=mybir.AluOpType.add)
                                nc.sync.dma_start(out[rows, ns], ot[:])
    return nc


def host_prep(x, Wqkv, Wproj, rope_sin, rope_cos):
    xT = np.ascontiguousarray(x.reshape(BT, C).T.astype(np.float32))
    ang_sin = np.asarray(rope_sin, np.float32).T  # [32, T]
    ang_cos = np.asarray(rope_cos, np.float32).T
    CB = np.ascontiguousarray(np.tile(ang_cos, (4, 1)).astype(np.float32))
    sign = np.where((np.arange(128) % 64) < 32, -1.0, 1.0)[:, None]
    SB = np.ascontiguousarray((np.tile(ang_sin, (4, 1)) * sign).astype(np.float32))
    PERM = np.zeros((128, 128), np.float32)
    for r in range(128):
        s = r + 32 if (r % 64) < 32 else r - 32
        PERM[s, r] = 1.0
    TRI = (np.arange(128)[None, :] >= np.arange(128)[:, None]).astype(np.float32)
    TRI = np.ascontiguousarray(TRI)
    IDT = np.eye(128, dtype=np.float32)
    Wqkv = np.asarray(Wqkv, np.float32)
    Wproj = np.asarray(Wproj, np.float32)
    in_maps = []
    for i in range(N_CORES):
        hs = [HPC * i + j for j in range(HPC)]
        wq_ = np.concatenate([Wqkv[:, h * 192: h * 192 + 64] for h in hs], axis=1)
        wk_ = np.concatenate([Wqkv[:, h * 192 + 64: h * 192 + 128] for h in hs], axis=1)
        wv_ = np.concatenate([Wqkv[:, h * 192 + 128: h * 192 + 192] for h in hs], axis=1)
        wp_ = np.concatenate([Wproj[h * HD:(h + 1) * HD, :] for h in hs], axis=0)
        in_maps.append({
            "xT": xT, "wq": np.ascontiguousarray(wq_),
            "wk": np.ascontiguousarray(wk_), "wv": np.ascontiguousarray(wv_),
            "wp": np.ascontiguousarray(wp_), "cb": CB, "sb": SB,
            "perm": PERM, "tri": TRI, "idt": IDT,
            "ones": np.ones((128, HPC, NJ * 4, 64), np.float32)})
    return in_maps


_CACHE = {}


def _get_program():
    if "nc" not in _CACHE:
        nc = bacc.Bacc("TRN2", target_bir_lowering=False, debug=False,
                       num_devices=N_CORES)
        build_program(nc)
        nc.compile()
        _CACHE["nc"] = nc
    return _CACHE["nc"]


def kernel(x, Wqkv, Wproj, rope_sin, rope_cos):
    nc = _get_program()
    in_maps = host_prep(x, Wqkv, Wproj, rope_sin, rope_cos)
    res = bass_utils.run_bass_kernel_spmd(nc, in_maps,
                                          core_ids=list(range(N_CORES)))
    total = np.zeros((BT, C), np.float64)
    for i in range(N_CORES):
        total += res.results[i]["out"].astype(np.float64)
    return total.astype(np.float32).reshape(B, T, C)


# revision 44
# speedup vs baseline: 1.3743x; 1.3743x over previous
"""Trainium2 Bass kernel for nn_MultiHeadAttention_46471546143554.

Head-parallel sharding: 16 heads / 8 cores = 2 heads per core. Each core
computes QKV projection (its head slice), RoPE, causal attention, and a
per-head output projection producing a partial [B*T, C] sum; the host adds
the 8 partials.

Layout trick: everything runs "transposed" ([feature, token]) so the PE
contracts over partitions with zero on-device transposes of activations:
  qkvT = W.T @ xT          (xT passed pre-transposed from host)
  S^T  = kT.T @ qT         (per 128-key block)
  P^T  = exp(S^T * scale)  (no max subtraction; scores are O(+-8))
  A^T  = v_aug.T @ P^T     (v_aug = [v | ones] -> row 64 = softmax denom)
  out  = A^T.T @ Wp_head   (per head; divide by denom at PSUM eviction,
                            where the denom is a per-partition scalar)
"""
import numpy as np

import concourse.bass as bass
import concourse.mybir as mybir
import concourse.tile as tile
from concourse import bacc
from concourse import bass_utils

B, T, C = 2, 2048, 1024
H, HD, HALF = 16, 64, 32
BT = B * T
N_CORES = 8
HPC = 2              # heads per core
NKC = C // 128       # contraction chunks for projection
NJ = BT // 512       # 512-token blocks overall
NQ = T // 512        # tq blocks per batch
NKB = T // 128       # tk blocks per batch
ROPE_BASE = 10000.0

F32 = mybir.dt.float32
F32R = mybir.dt.float32r
BF16 = mybir.dt.bfloat16
OUT_DT = BF16        # partial-sum output dtype (host reduces in fp32)
MM_DT = F32R         # matmul streaming dtype (1 cycle/row when N>=256)
SDT = MM_DT          # storage dtype for tiles feeding matmuls
SCALE = float(HD) ** -0.5


def _mm(ap):
    return ap


def build_program(nc):
    xT = nc.dram_tensor("xT", [C, BT], SDT, kind="ExternalInput").ap()
    wq = nc.dram_tensor("wq", [C, 128], SDT, kind="ExternalInput").ap()
    wk = nc.dram_tensor("wk", [C, 128], SDT, kind="ExternalInput").ap()
    wv = nc.dram_tensor("wv", [C, 128], SDT, kind="ExternalInput").ap()
    wp = nc.dram_tensor("wp", [128, C], SDT, kind="ExternalInput").ap()
    cb = nc.dram_tensor("cb", [128, T], SDT, kind="ExternalInput").ap()
    sb = nc.dram_tensor("sb", [128, T], SDT, kind="ExternalInput").ap()
    perm = nc.dram_tensor("perm", [128, 128], SDT, kind="ExternalInput").ap()
    tri = nc.dram_tensor("tri", [128, 128], SDT, kind="ExternalInput").ap()
    idt = nc.dram_tensor("idt", [128, 128], SDT, kind="ExternalInput").ap()
    e1 = nc.dram_tensor("e1", [64, 128], SDT, kind="ExternalInput").ap()
    onesr = nc.dram_tensor("onesr", [1, 128], SDT, kind="ExternalInput").ap()
    ones = nc.dram_tensor("ones", [128, 40], SDT, kind="ExternalInput").ap()
    out = nc.dram_tensor("out", [BT, C], OUT_DT, kind="ExternalOutput").ap()

    STT = mybir.AluOpType
    EXP = mybir.ActivationFunctionType.Exp

    with tile.TileContext(nc) as tc:
        from contextlib import ExitStack
        with ExitStack() as ctx:
            const = ctx.enter_context(tc.tile_pool(name="const", bufs=1))
            persist = ctx.enter_context(tc.tile_pool(name="persist", bufs=1))

            wq_s = const.tile([128, NKC, 128], SDT, tag="wq")
            wk_s = const.tile([128, NKC, 128], SDT, tag="wk")
            wv_s = const.tile([128, NKC, 128], SDT, tag="wv")
            wp_s = const.tile([64, HPC, C], SDT, tag="wp")
            cb_s = const.tile([128, T], SDT, tag="cb")
            sb_s = const.tile([128, T], SDT, tag="sb")
            perm_s = const.tile([128, 128], SDT, tag="perm")
            tri_s = const.tile([128, 128], SDT, tag="tri")
            idt_s = const.tile([128, 128], SDT, tag="idt")
            e1_s = const.tile([64, 128], SDT, tag="e1")
            onesr_s = const.tile([65, 128], SDT, tag="onesr")
            # weights first so the first matmuls can start ASAP
            nc.sync.dma_start(wq_s[:],
                              wq.rearrange("(kc p) m -> p kc m", p=128))

            qT_s = persist.tile([128, BT], SDT, tag="qT")
            kT_s = persist.tile([128, BT], SDT, tag="kT")
            vag_s = persist.tile([128, HPC, NJ * 4, 104], SDT, tag="vag")

            with (
                tc.tile_pool(name="xp", bufs=2) as xp,
                tc.tile_pool(name="evp", bufs=3) as evp,
                tc.tile_pool(name="rtmp", bufs=4) as rtmp,
                tc.tile_pool(name="pp", bufs=6) as pp,
                tc.tile_pool(name="rcp", bufs=3) as rcp,
                tc.tile_pool(name="rcbp", bufs=2) as rcbp,
                tc.tile_pool(name="atsp", bufs=2) as atsp,
                tc.tile_pool(name="ats2p", bufs=2) as ats2p,
                tc.tile_pool(name="otp", bufs=2) as otp,
                tc.tile_pool(name="projp", bufs=1, space="PSUM") as projp,
                tc.tile_pool(name="psS", bufs=2, space="PSUM") as psS,
                tc.tile_pool(name="psAT", bufs=2, space="PSUM") as psAT,
                tc.tile_pool(name="psRCB", bufs=1, space="PSUM") as psRCB,
                tc.tile_pool(name="flexB", bufs=2, space="PSUM") as flexB,
            ):
                for j in range(NJ):
                    b, jq = j // NQ, j % NQ
                    js = slice(j * 512, (j + 1) * 512)
                    rs_ = slice(jq * 512, (jq + 1) * 512)  # rope cols
                    # ---------- x strips (lookahead prefetch) ----------
                    if j == 0:
                        cur_a = xp.tile([128, NKC // 2, 512], SDT, tag="xsa")
                        cur_b = xp.tile([128, NKC // 2, 512], SDT, tag="xsb")
                        nc.sync.dma_start(
                            cur_a[:],
                            xT[0:512, js].rearrange("(kc p) t -> p kc t", p=128))
                        nc.sync.dma_start(
                            cur_b[:],
                            xT[512:1024, js].rearrange("(kc p) t -> p kc t",
                                                       p=128))
                        nc.sync.dma_start(
                            wk_s[:], wk.rearrange("(kc p) m -> p kc m", p=128))
                        nc.sync.dma_start(
                            wv_s[:], wv.rearrange("(kc p) m -> p kc m", p=128))
                        nc.sync.dma_start(idt_s[:], idt[:])
                        nc.sync.dma_start(perm_s[:], perm[:])
                        nc.sync.dma_start(cb_s[:], cb[:])
                        nc.sync.dma_start(sb_s[:], sb[:])
                        nc.sync.dma_start(tri_s[:], tri[:])
                        for _h in range(HPC):
                            nc.sync.dma_start(
                                vag_s[:, _h, :, 64:104],
                                ones[:, None, :].broadcast_to(
                                    (128, NJ * 4, 40)))
                        nc.sync.dma_start(onesr_s[64:65, :], onesr[:])
                        nc.sync.dma_start(e1_s[:], e1[:])
                        nc.sync.dma_start(wp_s[:], wp.rearrange("(h p) c -> p h c", h=HPC))
                    else:
                        cur_a, cur_b = next_a, next_b
                    xtiles = ([cur_a[:, kc, :] for kc in range(NKC // 2)]
                              + [cur_b[:, kc, :] for kc in range(NKC // 2)])
                    if j + 1 < NJ:
                        njs = slice((j + 1) * 512, (j + 2) * 512)
                        next_a = xp.tile([128, NKC // 2, 512], SDT, tag="xsa")
                        next_b = xp.tile([128, NKC // 2, 512], SDT, tag="xsb")
                        nc.sync.dma_start(
                            next_a[:],
                            xT[0:512, njs].rearrange("(kc p) t -> p kc t",
                                                     p=128))
                        nc.sync.dma_start(
                            next_b[:],
                            xT[512:1024, njs].rearrange("(kc p) t -> p kc t",
                                                        p=128))
                    # ---------- projections (serial q, k, v) ----------
                    for which, w_s in (("q", wq_s), ("k", wk_s), ("v", wv_s)):
                        ps_p = projp.tile([128, 512], F32, tag="proj")
                        for kc in range(NKC):
                            nc.tensor.matmul(ps_p[:], w_s[:, kc, :], xtiles[kc],
                                             start=(kc == 0),
                                             stop=(kc == NKC - 1))
                        if which == "v":
                            vtmp = evp.tile([128, 512], SDT, tag="vtmp")
                            nc.vector.tensor_copy(vtmp[:], ps_p[:])
                            for h in range(HPC):
                                for t4 in range(4):
                                    ps_vt = flexB.tile([128, 64], SDT,
                                                       tag="flexB")
                                    nc.tensor.transpose(
                                        ps_vt[:],
                                        vtmp[h * 64:(h + 1) * 64,
                                             t4 * 128:(t4 + 1) * 128],
                                        idt_s[h * 64:(h + 1) * 64,
                                              h * 64:(h + 1) * 64])
                                    nc.vector.tensor_copy(
                                        vag_s[:, h, j * 4 + t4, 0:64],
                                        ps_vt[:])
                        else:
                            dstT = qT_s if which == "q" else kT_s
                            raw = evp.tile([128, 512], SDT, tag="raw")
                            nc.vector.tensor_copy(raw[:], ps_p[:])
                            ps_sw = flexB.tile([128, 512], F32, tag="flexB")
                            nc.tensor.matmul(ps_sw[:], perm_s[:], raw[:],
                                             start=True, stop=True)
                            t1 = rtmp.tile([128, 512], SDT, tag="t1")
                            t2 = rtmp.tile([128, 512], SDT, tag="t2")
                            nc.vector.tensor_mul(t1[:], ps_sw[:], sb_s[:, rs_])
                            nc.gpsimd.tensor_mul(t2[:], raw[:], cb_s[:, rs_])
                            nc.vector.tensor_add(dstT[:, js], t1[:], t2[:])
                    # ---------- attention for (b, jq) ----------
                    atsl = []
                    for h in range(HPC):
                        hs = slice(h * 64, (h + 1) * 64)
                        ps_at = psAT.tile([128, 512], F32, tag="ps_at")
                        nkb = 4 * jq + 4
                        for kb in range(nkb):
                            kcols = slice(b * T + kb * 128,
                                          b * T + (kb + 1) * 128)
                            c0 = max((kb - 4 * jq) * 128, 0)
                            qcols_t = slice(b * T + jq * 512 + c0,
                                            b * T + (jq + 1) * 512)
                            ps_s = psS.tile([128, 512], F32, tag="ps_s")
                            nc.tensor.matmul(ps_s[:, c0:512],
                                             kT_s[hs, kcols],
                                             qT_s[hs, qcols_t],
                                             start=True, stop=True)
                            pt = pp.tile([128, 512], SDT, tag="pt")
                            nc.scalar.activation(pt[:, c0:512], ps_s[:, c0:512],
                                                 EXP, scale=SCALE)
                            if kb >= 4 * jq:
                                nc.gpsimd.tensor_mul(
                                    pt[:, c0:c0 + 128], pt[:, c0:c0 + 128],
                                    tri_s[:])
                            nc.tensor.matmul(
                                ps_at[0:104, c0:512],
                                vag_s[:, h, b * 16 + kb, :],
                                pt[:, c0:512],
                                start=(kb == 0), stop=(kb == nkb - 1))
                        # softmax denom -> broadcast reciprocal to all rows
                        recipT = rcp.tile([65, 512], SDT, tag="recipT")
                        with nc.allow_low_precision(
                                reason="f32r recip of softmax denom"):
                            nc.vector.reciprocal(recipT[64:65, :],
                                                 ps_at[64:65, :])
                        ps_rcb = psRCB.tile([128, 512], F32, tag="psrcb")
                        nc.tensor.matmul(ps_rcb[:], onesr_s[64:65, :],
                                         recipT[64:65, :],
                                         start=True, stop=True)
                        rcbs = rcbp.tile([64, 512], SDT, tag="rcbs")
                        nc.vector.tensor_copy(rcbs[:], ps_rcb[0:64, :])
                        ats_h = atsp.tile([64, 512], SDT, tag="ats_h")
                        nc.vector.tensor_mul(ats_h[:], ps_at[0:64, :],
                                             rcbs[:])
                        atsl.append(ats_h)
                    # ---------- output projection (heads pre-scaled) ----------
                    for t4h in range(2):
                        ot = otp.tile([128, 2, C], OUT_DT, tag="ot")
                        for t4i in range(2):
                            t4 = t4h * 2 + t4i
                            for n2 in range(2):
                                ns = slice(n2 * 512, (n2 + 1) * 512)
                                ps_o = flexB.tile([128, 512], F32, tag="flexB")
                                for h in range(HPC):
                                    nc.tensor.matmul(
                                        ps_o[:],
                                        atsl[h][:, t4 * 128:(t4 + 1) * 128],
                                        wp_s[:, h, ns],
                                        start=(h == 0), stop=(h == 1))
                                if n2 == 0:
                                    nc.vector.tensor_copy(ot[:, t4i, ns],
                                                          ps_o[:])
                                else:
                                    nc.scalar.copy(ot[:, t4i, ns], ps_o[:])
                        orows = out[b * T + jq * 512 + t4h * 256:
                                    b * T + jq * 512 + (t4h + 1) * 256, :]
                        nc.scalar.dma_start(
                            orows.rearrange("(r p) c -> p r c", p=128), ot[:])
    return nc


def host_prep(x, Wqkv, Wproj, rope_sin, rope_cos):
    xT = np.ascontiguousarray(x.reshape(BT, C).T.astype(np.float32))
    ang_sin = np.asarray(rope_sin, np.float32).T  # [32, T]
    ang_cos = np.asarray(rope_cos, np.float32).T
    CB = np.ascontiguousarray(np.tile(ang_cos, (4, 1)).astype(np.float32))
    sign = np.where((np.arange(128) % 64) < 32, -1.0, 1.0)[:, None]
    SB = np.ascontiguousarray((np.tile(ang_sin, (4, 1)) * sign).astype(np.float32))
    PERM = np.zeros((128, 128), np.float32)
    for r in range(128):
        s = r + 32 if (r % 64) < 32 else r - 32
        PERM[s, r] = 1.0
    TRI = (np.arange(128)[None, :] >= np.arange(128)[:, None]).astype(np.float32)
    TRI = np.ascontiguousarray(TRI)
    IDT = np.eye(128, dtype=np.float32)
    E1 = np.zeros((64, 128), np.float32)
    E1[np.arange(64), 64 + np.arange(64)] = 1.0
    Wqkv = np.asarray(Wqkv, np.float32)
    Wproj = np.asarray(Wproj, np.float32)
    in_maps = []
    for i in range(N_CORES):
        hs = [HPC * i + j for j in range(HPC)]
        wq_ = np.concatenate([Wqkv[:, h * 192: h * 192 + 64] for h in hs], axis=1)
        wk_ = np.concatenate([Wqkv[:, h * 192 + 64: h * 192 + 128] for h in hs], axis=1)
        wv_ = np.concatenate([Wqkv[:, h * 192 + 128: h * 192 + 192] for h in hs], axis=1)
        wp_ = np.concatenate([Wproj[h * HD:(h + 1) * HD, :] for h in hs], axis=0)
        # wp_ rows: [h0(64), h1(64)] -> used as rhs [128, C] directly
        in_maps.append({
            "xT": xT, "wq": np.ascontiguousarray(wq_),
            "wk": np.ascontiguousarray(wk_), "wv": np.ascontiguousarray(wv_),
            "wp": np.ascontiguousarray(wp_), "cb": CB, "sb": SB,
            "perm": PERM, "tri": TRI, "idt": IDT, "e1": E1,
            "onesr": np.ones((1, 128), np.float32),
            "ones": np.ones((128, 40), np.float32)})
    return in_maps


_CACHE = {}


def _get_program():
    if "nc" not in _CACHE:
        nc = bacc.Bacc("TRN2", target_bir_lowering=False, debug=False,
                       num_devices=N_CORES)
        build_program(nc)
        nc.compile()
        _CACHE["nc"] = nc
    return _CACHE["nc"]


def kernel(x, Wqkv, Wproj, rope_sin, rope_cos):
    nc = _get_program()
    in_maps = host_prep(x, Wqkv, Wproj, rope_sin, rope_cos)
    res = bass_utils.run_bass_kernel_spmd(nc, in_maps,
                                          core_ids=list(range(N_CORES)))
    total = np.zeros((BT, C), np.float64)
    for i in range(N_CORES):
        total += res.results[i]["out"].astype(np.float64)
    return total.astype(np.float32).reshape(B, T, C)
